# revision 1
# baseline (speedup 1.0000x reference)
"""Disentangled multi-head attention (DeBERTa-style) on 8 Trainium2 NeuronCores.

Sharding: core c -> batch b = c // 4, head group g = c % 4 (4 of 16 heads).
Each core computes its 4 heads end-to-end (column-parallel QKV projections,
attention, row-parallel slice of the output projection); the host sums the
4 partial outputs per batch in fp32 and adds the bias terms.

Math folds (exact up to bf16 rounding):
  - scores = (q_c.(k_c+k_p) + q_p.k_c) * s as ONE K=128 matmul per tile with
    qcat = [q_c*s ; q_p*s], kcat = [k_c+k_p ; k_c] (scale folded into weights,
    weights concatenated per head host-side).
  - k_c+k_p via stacked contraction over [k.T ; pos_k.T] with [Wk.T ; Wpk.T].
  - gate: Wg*(1/s) replicated across 128 stationary columns -> the matmul
    with q_c*s yields the partition-broadcast pre-activation directly;
    Sigmoid(+bg) on ACT emits the bf16 gate tile.
  - gate*spatial_bias accumulated into score PSUM via identity matmul.
  - softmax without max-subtraction (scores bounded ~+-8, fp32-exp safe).
  - row-sums from a packed ones-column in the token-major ctx matmul, so
    normalization is per-partition reciprocal + tensor_scalar multiply.
  - bq/bpq/bk/bpk added per-partition during PSUM->SBUF copies; bv and bo
    folded on host (softmax rows sum to 1 when mask is all-True):
    attn @ (v+bv) @ Wo.T + bo = attn @ v @ Wo.T + (bv @ Wo.T + bo).
"""

import sys

sys.path.insert(0, "/opt/trn_rl_repo")

from contextlib import ExitStack

import numpy as np
import ml_dtypes

import concourse.bass as bass
from concourse import mybir, masks
from concourse.tile import TileContext
from concourse.bass_utils import run_bass_kernel_spmd

BF16 = ml_dtypes.bfloat16

B, L, D = 2, 2048, 1024
H = 16
HK = 64          # head dim
NCORES = 8
HPC = 4          # heads per core
CS = HPC * HK    # channels per core = 256
NJ = L // 128    # 16 key/token blocks
NCH = L // 512   # 4 query chunks
KB_D = D // 128  # 8 contraction blocks for a 1024-deep dim
KB_2D = 2 * KB_D
SCALE = float(1.0 / np.sqrt(HK))

_FP32 = mybir.dt.float32
_BF16 = mybir.dt.bfloat16
_EXP = mybir.ActivationFunctionType.Exp
_SIG = mybir.ActivationFunctionType.Sigmoid


def _split_multiwaits(nc, skip_opcodes=()):
    """This walrus build encodes at most one sync-wait per TPB instruction.
    Tile attaches several; hoist the extras onto same-engine NoOps placed
    immediately before the instruction (engines are in-order, so semantics
    are preserved)."""
    nsplit = 0
    for fn in nc.m.functions:
        for blk in fn.blocks:
            insts = blk.instructions
            out = []
            for inst in insts:
                si = inst.sync_info
                waits = list(si.on_wait) if si is not None and si.on_wait else []
                if len(waits) > 1 and inst.opcode not in skip_opcodes:
                    si.on_wait = waits[-1:]
                    for i, w in enumerate(waits[:-1]):
                        nop = mybir.InstNoOp(name=f"{inst.name}-w{i}",
                                             ins=[], outs=[])
                        nop.engine = inst.engine
                        nop.sync_info = type(si)(on_wait=[w], on_update=[])
                        out.append(nop)
                    nsplit += 1
                out.append(inst)
            if len(out) != len(insts):
                blk.instructions = out
    return nsplit


def build_nc():
    """Emit the per-core BIR (identical on all 8 cores; data differs)."""
    nc = bass.Bass()

    xq = nc.dram_tensor("xq", [2 * D, L], _BF16, kind="ExternalInput")
    xkk = nc.dram_tensor("xkk", [2 * D, L], _BF16, kind="ExternalInput")
    xv = nc.dram_tensor("xv", [D, L], _BF16, kind="ExternalInput")
    sbt = nc.dram_tensor("sbt", [L, L], _BF16, kind="ExternalInput")
    wqcat = nc.dram_tensor("wqcat", [2 * D, 512], _BF16, kind="ExternalInput")
    wkcat = nc.dram_tensor("wkcat", [2 * D, 512], _BF16, kind="ExternalInput")
    wv = nc.dram_tensor("wv", [D, CS], _BF16, kind="ExternalInput")
    wg8 = nc.dram_tensor("wg8", [HK, 128], _BF16, kind="ExternalInput")
    wo = nc.dram_tensor("wo", [CS, D], _BF16, kind="ExternalInput")
    pbq = nc.dram_tensor("pbq", [128, HPC], _FP32, kind="ExternalInput")
    pbk = nc.dram_tensor("pbk", [128, HPC], _FP32, kind="ExternalInput")
    g0 = nc.dram_tensor("g0", [128, HPC], _FP32, kind="ExternalInput")
    outT = nc.dram_tensor("outT", [D, L], _FP32, kind="ExternalOutput")

    with TileContext(nc) as tc, ExitStack() as top:
        pool = lambda **kw: top.enter_context(tc.tile_pool(**kw))

        const_pool = pool(name="const", bufs=1)
        w_pool = pool(name="w", bufs=1)
        bias_pool = pool(name="bias", bufs=1)
        x_pool = pool(name="xin", bufs=8)
        qk_pool = pool(name="qkres", bufs=1)
        v_pool = pool(name="vres", bufs=1)
        gb_pool = pool(name="gb", bufs=1)
        tmp_pool = pool(name="tmp", bufs=4)
        e_pool = pool(name="et", bufs=4)
        ctx_pool = pool(name="csb", bufs=4)
        inv_pool = pool(name="inv", bufs=4)
        cta_pool = pool(name="cta", bufs=1)
        oute_pool = pool(name="oute", bufs=6)

        ident = const_pool.tile([128, 128], _BF16, tag="ident", name="ident")
        masks.make_identity(nc, ident[:])

        def load_w(name, src, nkb, width, tag):
            ts = []
            for kb in range(nkb):
                t = w_pool.tile([128, width], _BF16, tag=f"{tag}{kb}", name=f"{tag}{kb}")
                nc.sync.dma_start(t[:], src[kb * 128:(kb + 1) * 128, :])
                ts.append(t)
            return ts

        wq_t = load_w("wqcat", wqcat, KB_2D, 512, "wq")
        wk_t = load_w("wkcat", wkcat, KB_2D, 512, "wk")
        wv_t = load_w("wv", wv, KB_D, CS, "wv")
        wo_t = load_w("wo", wo, 2, D, "wo")
        wg8_t = const_pool.tile([HK, 128], _BF16, tag="wg8", name="wg8t")
        nc.sync.dma_start(wg8_t[:], wg8[:, :])
        pbq_t = bias_pool.tile([128, HPC], _FP32, tag="pbq", name="pbqt")
        nc.sync.dma_start(pbq_t[:], pbq[:, :])
        pbk_t = bias_pool.tile([128, HPC], _FP32, tag="pbk", name="pbkt")
        nc.sync.dma_start(pbk_t[:], pbk[:, :])
        g0_t = bias_pool.tile([128, HPC], _FP32, tag="g0", name="g0t")
        nc.sync.dma_start(g0_t[:], g0[:, :])

        qcat = [qk_pool.tile([128, L], _BF16, tag=f"qcat{h}", name=f"qcat{h}") for h in range(HPC)]
        kcat = [qk_pool.tile([128, L], _BF16, tag=f"kcat{h}", name=f"kcat{h}") for h in range(HPC)]
        vones = [[None] * NJ for _ in range(HPC)]

        # ---- phase P1: v projection (token-major) -----------------------
        # xv k-rows live only here; its SBUF region is reused for sbt after.
        with tc.tile_pool(name="xv_rows", bufs=KB_D) as xvr_pool, \
             tc.tile_pool(name="ps_v", bufs=3, space="PSUM") as psv_pool:
            xv_t = []
            for kb in range(KB_D):
                t = xvr_pool.tile([128, L], _BF16, tag="xvr", name="xvr")
                nc.sync.dma_start(t[:], xv[kb * 128:(kb + 1) * 128, :])
                xv_t.append(t)
            for tb in range(NJ):
                ps = psv_pool.tile([128, CS], _FP32, tag="ps_v", name="psv")
                for kb in range(KB_D):
                    nc.tensor.matmul(
                        ps[:], xv_t[kb][:, tb * 128:(tb + 1) * 128], wv_t[kb][:],
                        start=(kb == 0), stop=(kb == KB_D - 1))
                for h in range(HPC):
                    vb = v_pool.tile([128, 65], _BF16, tag=f"vb{h}_{tb}", name=f"vb{h}_{tb}")
                    nc.vector.tensor_copy(vb[:, 0:HK], ps[:, h * HK:(h + 1) * HK])
                    nc.gpsimd.memset(vb[:, HK:65], 1.0)
                    vones[h][tb] = vb

        # spatial_bias.T resident tiles; DMAs overlap the q/kk projections.
        sbt_pool = pool(name="sbt", bufs=1)
        sbt_t = []
        for j in range(NJ):
            t = sbt_pool.tile([128, L], _BF16, tag=f"sbt{j}", name=f"sbtt{j}")
            nc.sync.dma_start(t[:], sbt[j * 128:(j + 1) * 128, :])
            sbt_t.append(t)

        # ---- phase P2: q/qp and kk/kc projections (channel-major) -------
        with tc.tile_pool(name="ps_p", bufs=HPC, space="PSUM") as psp_pool:
            for ch in range(NCH):
                csl = slice(ch * 512, (ch + 1) * 512)
                psq = [psp_pool.tile([128, 512], _FP32, tag="ps_p", name="psp")
                       for _ in range(HPC)]
                for kb in range(KB_2D):
                    xt = x_pool.tile([128, 512], _BF16, tag="xq", name="xqt")
                    nc.sync.dma_start(xt[:], xq[kb * 128:(kb + 1) * 128, csl])
                    for h in range(HPC):
                        nc.tensor.matmul(
                            psq[h][:], wq_t[kb][:, h * 128:(h + 1) * 128], xt[:],
                            start=(kb == 0), stop=(kb == KB_2D - 1))
                for h in range(HPC):
                    nc.vector.tensor_scalar_add(
                        qcat[h][:, csl], psq[h][:], pbq_t[:, h:h + 1])
            for ch in range(NCH):
                csl = slice(ch * 512, (ch + 1) * 512)
                psk = [psp_pool.tile([128, 512], _FP32, tag="ps_p", name="psp")
                       for _ in range(HPC)]
                for kb in range(KB_2D):
                    xt = x_pool.tile([128, 512], _BF16, tag="xkk", name="xkkt")
                    nc.sync.dma_start(xt[:], xkk[kb * 128:(kb + 1) * 128, csl])
                    for h in range(HPC):
                        nc.tensor.matmul(
                            psk[h][:], wk_t[kb][:, h * 128:(h + 1) * 128], xt[:],
                            start=(kb == 0), stop=(kb == KB_2D - 1))
                for h in range(HPC):
                    nc.vector.tensor_scalar_add(
                        kcat[h][:, csl], psk[h][:], pbk_t[:, h:h + 1])

        # ---- phases G/A/O share one PSUM layout: 3 + 4 + 1 banks --------
        pss_pool = pool(name="ps_s", bufs=3, space="PSUM")
        psctx_pool = pool(name="ps_ctx", bufs=4, space="PSUM")
        pst_pool = pool(name="ps_t", bufs=1, space="PSUM")

        # ---- phase G: all gate tiles (one Sigmoid batch, then Exp only) --
        gb = {}
        for h in range(HPC):
            for ch in range(NCH):
                psg = pss_pool.tile([128, 512], _FP32, tag="ps_s", name="pss")
                nc.tensor.matmul(psg[:], wg8_t[:],
                                 qcat[h][0:HK, ch * 512:(ch + 1) * 512])
                g = gb_pool.tile([128, 512], _BF16, tag=f"gb{h}_{ch}", name=f"gbt{h}_{ch}")
                nc.scalar.activation(g[:], psg[:], _SIG, bias=g0_t[:, h:h + 1])
                gb[(h, ch)] = g

        # ---- phase A: attention -----------------------------------------
        cta = [cta_pool.tile([128, L], _BF16, tag=f"cta{k}", name=f"cta{k}") for k in range(2)]
        for h in range(HPC):
            for ch in range(NCH):
                isl = slice(ch * 512, (ch + 1) * 512)
                pctx = [psctx_pool.tile([128, 65], _FP32, tag="ps_ctx", name="psctx")
                        for _ in range(4)]
                for j in range(NJ):
                    pss = pss_pool.tile([128, 512], _FP32, tag="ps_s", name="pss")
                    nc.tensor.matmul(pss[:], kcat[h][:, j * 128:(j + 1) * 128],
                                     qcat[h][:, isl], start=True, stop=False)
                    tmp = tmp_pool.tile([128, 512], _BF16, tag="tmp", name="tmpt")
                    nc.vector.tensor_mul(tmp[:], gb[(h, ch)][:], sbt_t[j][:, isl])
                    nc.tensor.matmul(pss[:], ident[:], tmp[:],
                                     start=False, stop=True)
                    et = e_pool.tile([128, 512], _BF16, tag="et", name="ett")
                    nc.scalar.activation(et[:], pss[:], _EXP)
                    for s in range(4):
                        nc.tensor.matmul(
                            pctx[s][:], et[:, s * 128:(s + 1) * 128],
                            vones[h][j][:],
                            start=(j == 0), stop=(j == NJ - 1))
                # normalize + transpose to channel-major ctxT
                rows = slice((h % 2) * HK, (h % 2) * HK + HK)
                for s in range(4):
                    inv = inv_pool.tile([128, 1], _FP32, tag="inv", name="invt")
                    nc.vector.reciprocal(inv[:], pctx[s][:, HK:65])
                    csb = ctx_pool.tile([128, HK], _BF16, tag="csb", name="csbt")
                    nc.vector.tensor_scalar_mul(csb[:], pctx[s][:, 0:HK], inv[:])
                    pt = pst_pool.tile([128, 128], _BF16, tag="ps_t", name="pstt")
                    nc.tensor.matmul(pt[rows, :], csb[:], ident[:],
                                     is_transpose=True)
                    col = ch * 512 + s * 128
                    nc.vector.tensor_copy(cta[h // 2][rows, col:col + 128],
                                          pt[rows, :])

        # ---- phase O: output projection (row-parallel slice) ------------
        for ob in range(KB_D):
            for ch in range(NCH):
                ps = pss_pool.tile([128, 512], _FP32, tag="ps_s", name="pss")
                for kb in range(2):
                    nc.tensor.matmul(
                        ps[:], wo_t[kb][:, ob * 128:(ob + 1) * 128],
                        cta[kb][:, ch * 512:(ch + 1) * 512],
                        start=(kb == 0), stop=(kb == 1))
                ot = oute_pool.tile([128, 512], _FP32, tag="ot", name="ott")
                nc.vector.tensor_copy(ot[:], ps[:])
                nc.sync.dma_start(
                    outT[ob * 128:(ob + 1) * 128, ch * 512:(ch + 1) * 512],
                    ot[:])

    _split_multiwaits(nc)
    return nc


_NC_CACHE = {}


def _get_nc():
    if "nc" not in _NC_CACHE:
        _NC_CACHE["nc"] = build_nc()
    return _NC_CACHE["nc"]


def _np_reference(k, v, q, mask, spatial_bias, pos_k, pos_q,
                  Wk, bk, Wv, bv, Wq, bq, Wpk, bpk, Wpq, bpq, Wo, bo, Wg, bg):
    """Slow numpy fallback (only if mask is not all-True)."""
    def lin(x, W, b):
        return x @ W.T + b

    def split(x):
        return x.reshape(B, L, H, -1).transpose(0, 2, 1, 3)

    k_c, v_c, q_c = split(lin(k, Wk, bk)), split(lin(v, Wv, bv)), split(lin(q, Wq, bq))
    k_p, q_p = split(lin(pos_k, Wpk, bpk)), split(lin(pos_q, Wpq, bpq))
    scores = (np.einsum("bhqd,bhkd->bhqk", q_c, k_c)
              + np.einsum("bhqd,bhkd->bhqk", q_c, k_p)
              + np.einsum("bhqd,bhkd->bhqk", q_p, k_c)) * SCALE
    gate = 1.0 / (1.0 + np.exp(-(q_c @ Wg.T + bg)))
    scores = scores + gate * spatial_bias
    scores = np.where(mask[:, None, :, :], scores, -np.inf)
    scores = scores - scores.max(-1, keepdims=True)
    e = np.exp(scores)
    attn = e / e.sum(-1, keepdims=True)
    ctx = np.einsum("bhqk,bhkd->bhqd", attn, v_c)
    ctx = ctx.transpose(0, 2, 1, 3).reshape(B, L, D)
    return lin(ctx, Wo, bo).astype(np.float32)


def kernel(k, v, q, mask, spatial_bias, pos_k, pos_q,
           Wk, bk, Wv, bv, Wq, bq, Wpk, bpk, Wpq, bpq, Wo, bo, Wg, bg,
           **_unused):
    f32 = lambda x: np.asarray(x, np.float32)
    k, v, q, pos_k, pos_q = f32(k), f32(v), f32(q), f32(pos_k), f32(pos_q)
    spatial_bias = f32(spatial_bias)
    mask = np.asarray(mask)
    Wk, Wv, Wq, Wpk, Wpq, Wo, Wg = map(f32, (Wk, Wv, Wq, Wpk, Wpq, Wo, Wg))
    bk, bv, bq, bpk, bpq, bo, bg = map(f32, (bk, bv, bq, bpk, bpq, bo, bg))

    if not mask.all():
        return _np_reference(k, v, q, mask, spatial_bias, pos_k, pos_q,
                             Wk, bk, Wv, bv, Wq, bq, Wpk, bpk, Wpq, bpq,
                             Wo, bo, Wg, bg)

    nc = _get_nc()

    def t_bf16(x):  # [L, D] -> [D, L] bf16
        return np.ascontiguousarray(x.T).astype(BF16)

    xq_b = [np.ascontiguousarray(
        np.vstack([q[b].T, pos_q[b].T])).astype(BF16) for b in range(B)]
    xkk_b = [np.ascontiguousarray(
        np.vstack([k[b].T, pos_k[b].T])).astype(BF16) for b in range(B)]
    xv_b = [t_bf16(v[b]) for b in range(B)]
    sbt_b = [np.ascontiguousarray(spatial_bias[b, 0].T).astype(BF16)
             for b in range(B)]

    WqT, WpqT = Wq.T * SCALE, Wpq.T * SCALE
    WkT, WpkT, WvT, WoT = Wk.T, Wpk.T, Wv.T, Wo.T
    in_maps = []
    for c in range(NCORES):
        b, g = c // 4, c % 4
        cs = slice(g * CS, (g + 1) * CS)
        wqc = np.zeros((2 * D, 512), np.float32)
        wkc = np.empty((2 * D, 512), np.float32)
        pbq_a = np.empty((128, HPC), np.float32)
        pbk_a = np.empty((128, HPC), np.float32)
        for h in range(HPC):
            chs = slice(g * CS + h * HK, g * CS + (h + 1) * HK)
            wqc[0:D, h * 128:h * 128 + 64] = WqT[:, chs]
            wqc[D:, h * 128 + 64:(h + 1) * 128] = WpqT[:, chs]
            wkc[0:D, h * 128:h * 128 + 64] = WkT[:, chs]
            wkc[D:, h * 128:h * 128 + 64] = WpkT[:, chs]
            wkc[0:D, h * 128 + 64:(h + 1) * 128] = WkT[:, chs]
            wkc[D:, h * 128 + 64:(h + 1) * 128] = 0.0
            pbq_a[0:64, h] = bq[chs] * SCALE
            pbq_a[64:128, h] = bpq[chs] * SCALE
            pbk_a[0:64, h] = bk[chs] + bpk[chs]
            pbk_a[64:128, h] = bk[chs]
        wg8_a = np.repeat((Wg[0] * (1.0 / SCALE))[:, None], 128, axis=1)
        in_maps.append({
            "xq": xq_b[b], "xkk": xkk_b[b], "xv": xv_b[b], "sbt": sbt_b[b],
            "wqcat": wqc.astype(BF16), "wkcat": wkc.astype(BF16),
            "wv": np.ascontiguousarray(WvT[:, cs]).astype(BF16),
            "wg8": wg8_a.astype(BF16),
            "wo": np.ascontiguousarray(WoT[cs, :]).astype(BF16),
            "pbq": pbq_a, "pbk": pbk_a,
            "g0": np.full((128, HPC), float(bg[0]), np.float32),
        })

    res = run_bass_kernel_spmd(nc, in_maps, core_ids=list(range(NCORES)))

    const_row = (bv @ WoT + bo).astype(np.float32)  # exact bv/bo fold
    out = np.empty((B, L, D), np.float32)
    for b in range(B):
        acc = res.results[b * 4]["outT"].astype(np.float32, copy=True)
        for g in range(1, 4):
            acc += res.results[b * 4 + g]["outT"]
        out[b] = acc.T + const_row
    return out

